# revision 1
# baseline (speedup 1.0000x reference)
"""Trainium2 Bass kernel for a LocallyConnected1D layer.

Reference computation (fp32):
    x:      (B=64, L=256, C=192)
    kernel: (out_len=254, K*C=576, F=192)   per-position (unshared) weights
    bias:   (out_len=254, F=192)
    out[b, l, f] = sum_k patches[b, l, k] * kernel[l, k, f] + bias[l, f]
    where patches[b, l, :] = x[b, l:l+3, :].reshape(576)

Because x[b, l:l+3, :].ravel() == x[b].ravel()[192*l : 192*l + 576], the patch
tensor is just overlapping windows of the flattened x — no im2col needed.

Strategy: shard the output-position axis across the 8 NeuronCores (weights
dominate: 112 MB streamed exactly once; per-core slice ~14 MB).  Each core
computes 32 positions (cores pad the tail beyond 254 with zero weights).  Per
position: a (64x576)@(576x192) GEMM accumulated in PSUM as 4x K=128 + 1x K=64
matmuls with the batch dim as the stationary operand (M=64), plus a fused
bias-add during the PSUM->SBUF copy on the vector engine.

The host pre-transposes each core's x window into the [K, B] layout the PE
array needs (1.7 MB/core — ~1% of the weight traffic).
"""

import sys

sys.path.insert(0, "/opt/trn_rl_repo")

import numpy as np

import concourse.bass as bass
import concourse.mybir as mybir
import concourse.tile as tile
from concourse import bacc
from concourse.bass_utils import run_bass_kernel_spmd

# Problem constants (hardcoded per contract)
B = 64          # batch
L = 256         # input length
C = 192         # channels
KSZ = 3         # conv kernel size
F = 192         # output features
OUT_LEN = 254   # (L - KSZ) + 1
N_CORES = 8
P_CORE = 32     # positions per core (8*32 = 256 >= 254, tail padded)
KDIM = KSZ * C  # 576 contraction size per position

# per-core x window: positions p in [0,32) need flat-k in [192p, 192p+576)
# -> k span = 192*31 + 576 = 6528 = 51 * 128
XT_TILES = 51           # 128-row k-tiles of the transposed x window
XT_FREE = XT_TILES * B  # 3264
GROUP = 4               # positions per weight DMA group (4*576 = 2304 = 18*128)
N_GROUPS = P_CORE // GROUP
WT_BLKS = GROUP * KDIM // 128  # 18

DT = mybir.dt.float32

_cache = {}


def _build_program():
    """Build the per-core SPMD Bass program (identical on all 8 cores)."""
    nc = bacc.Bacc("TRN2", target_bir_lowering=False, debug=False,
                   num_devices=N_CORES)

    xt_d = nc.dram_tensor("xt", [128, XT_FREE], DT, kind="ExternalInput").ap()
    w_d = nc.dram_tensor("w", [P_CORE, KDIM, F], DT, kind="ExternalInput").ap()
    b_d = nc.dram_tensor("b", [1, P_CORE * F], DT, kind="ExternalInput").ap()
    out_d = nc.dram_tensor("out", [B, P_CORE, F], DT, kind="ExternalOutput").ap()

    with tile.TileContext(nc) as tc:
        with (
            tc.tile_pool(name="const", bufs=1) as cpool,
            tc.tile_pool(name="wt", bufs=3) as wpool,
            tc.tile_pool(name="osb", bufs=2) as opool,
            tc.tile_pool(name="ps", bufs=4, space="PSUM") as pspool,
        ):
            xt_sb = cpool.tile([128, XT_FREE], DT)
            nc.sync.dma_start(xt_sb[:], xt_d[:])

            bias_rep = cpool.tile([B, P_CORE * F], DT)
            nc.gpsimd.dma_start(bias_rep[:], b_d.to_broadcast((B, P_CORE * F)))

            for g in range(N_GROUPS):
                wt = wpool.tile([128, WT_BLKS * F], DT, tag="wt")
                src = (w_d[GROUP * g : GROUP * (g + 1)]
                       .rearrange("a b f -> (a b) f")
                       .rearrange("(d p) f -> p d f", p=128))
                nc.sync.dma_start(wt[:].rearrange("p (d f) -> p d f", d=WT_BLKS),
                                  src)

                osb = opool.tile([B, GROUP * F], DT, tag="osb")
                for pl in range(GROUP):
                    p = GROUP * g + pl
                    # (part_base, K, xt_free_tile_j, w_free_blk_d) per matmul
                    ops = []
                    if p % 2 == 0:
                        for i in range(4):
                            kpos = 3 * p + 2 * i
                            r0 = KDIM * pl + 128 * i
                            ops.append((0, 128, kpos // 2, r0 // 128))
                        ops.append((0, 64, (3 * p + 8) // 2,
                                    (KDIM * pl + 512) // 128))
                    else:
                        ops.append((64, 64, (3 * p) // 2, (KDIM * pl) // 128))
                        for i in range(4):
                            kpos = 3 * p + 2 * i + 1
                            r0 = KDIM * pl + 64 * (2 * i + 1)
                            ops.append((0, 128, kpos // 2, r0 // 128))

                    ps = pspool.tile([B, F], DT, tag="ps")
                    for idx, (pb, k, j, d) in enumerate(ops):
                        nc.tensor.matmul(
                            ps[:, :],
                            xt_sb[pb : pb + k, B * j : B * (j + 1)],
                            wt[pb : pb + k, F * d : F * (d + 1)],
                            start=(idx == 0),
                            stop=(idx == len(ops) - 1),
                        )
                    # fused PSUM->SBUF copy + bias add on the vector engine
                    nc.vector.tensor_add(
                        osb[:, F * pl : F * (pl + 1)],
                        ps[:, :],
                        bias_rep[:, F * p : F * (p + 1)],
                    )

                nc.sync.dma_start(
                    out_d[:, GROUP * g : GROUP * (g + 1), :],
                    osb[:].rearrange("p (a f) -> p a f", a=GROUP),
                )

    nc.compile()
    return nc


def shard_inputs(x, kernel, bias):
    """Slice + lay out the full inputs into per-core input maps."""
    x = np.ascontiguousarray(x, dtype=np.float32)
    kernel = np.ascontiguousarray(kernel, dtype=np.float32)
    bias = np.ascontiguousarray(bias, dtype=np.float32)

    xflat = x.reshape(B, L * C)
    pad_k = N_CORES * P_CORE  # 256 padded positions
    # x window for the last core reaches k = 192*224 + 6528 = 49536
    need = (pad_k - P_CORE) * C + XT_TILES * 128
    xflat = np.pad(xflat, ((0, 0), (0, need - L * C)))

    w_pad = np.zeros((pad_k, KDIM, F), dtype=np.float32)
    w_pad[:OUT_LEN] = kernel
    b_pad = np.zeros((pad_k, F), dtype=np.float32)
    b_pad[:OUT_LEN] = bias

    in_maps = []
    for c in range(N_CORES):
        k0 = P_CORE * C * c
        xsl = xflat[:, k0 : k0 + XT_TILES * 128]           # (64, 6528)
        xt = np.ascontiguousarray(
            xsl.reshape(B, XT_TILES, 128).transpose(2, 1, 0)
        ).reshape(128, XT_FREE)
        in_maps.append({
            "xt": xt,
            "w": np.ascontiguousarray(w_pad[P_CORE * c : P_CORE * (c + 1)]),
            "b": np.ascontiguousarray(
                b_pad[P_CORE * c : P_CORE * (c + 1)].reshape(1, P_CORE * F)),
        })
    return in_maps


def unshard_output(results):
    full = np.concatenate([results[c]["out"] for c in range(N_CORES)], axis=1)
    return np.ascontiguousarray(full[:, :OUT_LEN, :])


def get_program():
    if "nc" not in _cache:
        _cache["nc"] = _build_program()
    return _cache["nc"]


def kernel(x, kernel, bias):
    nc = get_program()
    in_maps = shard_inputs(x, kernel, bias)
    res = run_bass_kernel_spmd(nc, in_maps, list(range(N_CORES)))
    return unshard_output(res.results)
